# revision 23
# baseline (speedup 1.0000x reference)
"""Causal self-attention (B=4, T=2048, C=768, H=12) on 8 trn2 NeuronCores.

Sharding: core c -> batch b = c//2, head-half hh = c%2 (6 heads per core).
Each core computes, for its (b, 6 heads): qkv projection, causal attention,
and a partial output projection (its heads' rows of W_proj). The host sums
the two partial outputs per batch and adds b_proj.

v5 over the 321us baseline:
  - all matmul operands float16 (full PE rate, fast 256B FWL weight
    loads; quantization ~5e-4 per tensor).
  - PV uses merged [v | ones] (even heads) / [ones | v] (odd heads)
    [128,128] stationaries: ONE M=128 matmul per head per block computes
    both O^T and the softmax denominator.
  - causal masking of diagonal 128-blocks folded INTO the S matmul
    accumulation group (extra N=128 matmul adds -30 above the diagonal;
    exp maps those to ~0) — no post-exp mask op.
  - softmax exp split: ACT exp for diagonal + 2/3 of clean blocks;
    Schraudolph bits-of-fp16 exp (one Vector tensor_scalar -> int16,
    bitcast fp16) for the remaining third, mean-centered so softmax
    cancels its bias.
  - HAM warmup: ~4us of dummy matmuls on an SBUF scratch right after
    the engine-start barrier, so the PE clock is at 8/8 (2.4 GHz)
    before the first real matmul instead of ramping mid-qkv.
  - fillers (v units, next pair's q/k units, previous chunk's proj
    t-tiles) are interleaved BETWEEN attention blocks, keeping the PE
    busy through exp latency and chunk-boundary normalize chains (no
    >3.4us PE idle windows -> no HAM re-throttle).
  - wa DMA split so the v-projection columns arrive after the q/k
    columns and x, which gate the start of attention.
  - output projection + output DMA inlined per chunk.

Layout: q^T/k^T [d, T] fp16; S^T [tk, tq] blocks, head A on partitions
0-63 / head B on 64-127; each head's softmax denominator lands on the
partitions opposite its O^T rows (normalize = reciprocal +
partition-swap DMA + multiply).
"""

import numpy as np

B, T, C = 4, 2048, 768
H = 12
D = C // H          # 64
HPC = 6             # heads per core
NP = 3              # head pairs per core
N_CORES = 8
TK = T // 128       # 16 tk tiles
NCH = T // 512      # 4 tq chunks
CT = C // 128       # 6 contraction tiles

SCH_A = 1477.319722   # 1024/ln2
SCH_B = 15301.086468  # 15*1024 - mean-centering constant

_cache = {}


def _build(has_bias):
    import concourse.tile as tile
    from concourse import bacc, mybir

    dt = mybir.dt
    f32 = dt.float32
    f16 = dt.float16
    i16 = dt.int16
    Exp = mybir.ActivationFunctionType.Exp
    Alu = mybir.AluOpType

    nc = bacc.Bacc("TRN2", target_bir_lowering=False, debug=False,
                   num_devices=N_CORES)

    xT_ap = nc.dram_tensor("xT", [C, T], f16, kind="ExternalInput").ap()
    wa_ap = nc.dram_tensor("wa", [C, 1152], f16, kind="ExternalInput").ap()
    wp_ap = nc.dram_tensor("wp", [HPC * D, C], f16, kind="ExternalInput").ap()
    nid_ap = nc.dram_tensor("nid", [128, 128], f16, kind="ExternalInput").ap()
    mkl_ap = nc.dram_tensor("mkl", [128, 128], f16, kind="ExternalInput").ap()
    if has_bias:
        ox_ap = nc.dram_tensor("ox", [1, T], f16, kind="ExternalInput").ap()
        wb_ap = nc.dram_tensor("wb", [1, 1152], f16, kind="ExternalInput").ap()
    out_ap = nc.dram_tensor("out", [T, C], f32, kind="ExternalOutput").ap()

    with tile.TileContext(nc) as tc:
        with tc.tile_pool(name="pers", bufs=1) as pers, \
             tc.tile_pool(name="pP", bufs=6) as pP, \
             tc.tile_pool(name="pst", bufs=2) as pst, \
             tc.tile_pool(name="pout", bufs=2) as pout, \
             tc.tile_pool(name="psA", bufs=2, space="PSUM") as psA, \
             tc.tile_pool(name="psB", bufs=2, space="PSUM") as psB, \
             tc.tile_pool(name="psV", bufs=2, space="PSUM") as psV:

            # ---- HAM warmup: PE activity with no DMA dependency ----
            scr = pers.tile([128, 512], f16, tag="scr")
            nc.vector.memset(scr[:], 0.001)
            wps = psA.tile([128, 1024], f32, tag="A")
            for w in range(14):
                nc.tensor.matmul(wps[:, 0:512], lhsT=scr[:, 0:128],
                                 rhs=scr[:], start=True, stop=True)

            # ---- persistent SBUF tensors + input DMA ----
            # order = arrival order: mask constants (tiny), then x / wa-qk
            # interleaved (gate attention start), wp + wa-v last.
            nid = pers.tile([128, 128], f16, tag="nid")
            nc.sync.dma_start(nid[:], nid_ap)
            # mk2 = [mkl | mkl] so one N=256 matmul masks both heads'
            # diagonal sub-blocks (2-region strided PSUM out).
            mk2 = pers.tile([128, 256], f16, tag="mk2")
            nc.sync.dma_start(mk2[:, 0:128], mkl_ap)
            nc.sync.dma_start(mk2[:, 128:256], mkl_ap)
            xT = [pers.tile([128, T], f16, tag=f"x{i}", name=f"x{i}") for i in range(CT)]
            wa = [pers.tile([128, 1152], f16, tag=f"w{i}", name=f"w{i}") for i in range(CT)]
            for i in range(CT):
                nc.sync.dma_start(xT[i][:], xT_ap[i * 128:(i + 1) * 128, :])
                nc.sync.dma_start(wa[i][:, 0:768],
                                  wa_ap[i * 128:(i + 1) * 128, 0:768])
                nc.sync.dma_start(wa[i][:, 768:1152],
                                  wa_ap[i * 128:(i + 1) * 128, 768:1152])
            wp = [pers.tile([128, C], f16, tag=f"p{i}", name=f"wp{i}") for i in range(NP)]
            for i in range(NP):
                nc.sync.dma_start(wp[i][:], wp_ap[i * 128:(i + 1) * 128, :])
            if has_bias:
                ox = pers.tile([1, T], f16, tag="ox")
                nc.sync.dma_start(ox[:], ox_ap)
                wb = pers.tile([1, 1152], f16, tag="wb")
                nc.sync.dma_start(wb[:], wb_ap)

            qk = [pers.tile([128, T], f16, tag=f"qk{i}", name=f"qk{i}") for i in range(2 * NP)]
            # PV stationaries: [128,128] per (t-tile, head): v in one column
            # half, ones in the other (memset once on the idle gpsimd).
            vo = [[pers.tile([128, 128], f16, tag=f"v{t}_{h}", name=f"v{t}_{h}")
                   for h in range(HPC)] for t in range(TK)]
            for t in range(TK):
                for h in range(HPC):
                    oc = 64 if h % 2 == 0 else 0
                    nc.gpsimd.memset(vo[t][h][:, oc:oc + 64], 1.0)
            # Y^T for pair p: fresh tile for p=0; pairs 1 and 2 reuse the
            # q^T tiles of pairs 0 and 1, which are dead by the time attn
            # p starts writing (Tile tracks the WAR dependency).
            y0 = pers.tile([128, T], f16, tag="y0", name="y0")
            yt = [y0, qk[0], qk[2]]

            # ---- qkv projection / proj emit units (used as fillers) ----
            def emit_qk_unit(p, u, on_act=False):
                qsel, cp2 = u // 4, u % 4      # qsel: 0=q 1=k; cp2: 512-col blk
                dst = qk[2 * p + qsel]
                wcol = qsel * 384 + p * 128
                t0 = cp2 * 512
                ps = psB.tile([128, 512], f32, tag="B")
                for c in range(CT):
                    nc.tensor.matmul(
                        ps[:], lhsT=wa[c][:, wcol:wcol + 128],
                        rhs=xT[c][:, t0:t0 + 512],
                        start=(c == 0),
                        stop=(c == CT - 1 and not has_bias))
                if has_bias:
                    nc.tensor.matmul(
                        ps[:], lhsT=wb[0:1, wcol:wcol + 128],
                        rhs=ox[0:1, t0:t0 + 512],
                        start=False, stop=True)
                if on_act:
                    nc.scalar.copy(dst[:, t0:t0 + 512], ps[:])
                else:
                    nc.vector.tensor_copy(dst[:, t0:t0 + 512], ps[:])

            def emit_v_unit(t):
                ps = psB.tile([128, 512], f32, tag="B")
                for c in range(CT):
                    nc.tensor.matmul(
                        ps[:, 0:384],
                        lhsT=xT[c][:, t * 128:(t + 1) * 128],
                        rhs=wa[c][:, 768:1152],
                        start=(c == 0),
                        stop=(c == CT - 1 and not has_bias))
                if has_bias:
                    nc.tensor.matmul(
                        ps[:, 0:384],
                        lhsT=ox[0:1, t * 128:(t + 1) * 128],
                        rhs=wb[0:1, 768:1152],
                        start=False, stop=True)
                for h in range(HPC):
                    vc = 0 if h % 2 == 0 else 64
                    nc.vector.tensor_copy(
                        vo[t][h][:, vc:vc + 64], ps[:, h * 64:(h + 1) * 64])

            def emit_proj_t(t):
                ob = pout.tile([128, C], f32, tag="o")
                for (n0, n1) in ((0, 512), (512, 768)):
                    pp = psB.tile([128, 512], f32, tag="B")
                    for kk in range(NP):
                        nc.tensor.matmul(
                            pp[:, 0:n1 - n0],
                            lhsT=yt[kk][:, t * 128:(t + 1) * 128],
                            rhs=wp[kk][:, n0:n1],
                            start=(kk == 0), stop=(kk == NP - 1))
                    nc.vector.tensor_copy(ob[:, n0:n1], pp[:, 0:n1 - n0])
                nc.sync.dma_start(out_ap[t * 128:(t + 1) * 128, :], ob[:])

            def dummy_mms(n):
                # PE keep-warm filler: no data deps, writes a scratch slot.
                wd = psA.tile([128, 1024], f32, tag="A")
                for w in range(n):
                    nc.tensor.matmul(wd[:, 0:512], lhsT=scr[:, 0:128],
                                     rhs=scr[:], start=True, stop=True)

            # ---- phase 1 head: pair-0 q/k + first v tiles ----
            # dummy matmuls interleave with the DMA-paced units so the PE
            # never idles >3.4us (which would re-throttle HAM to 1.2 GHz).
            with nc.named_scope("qkv_qk"):
                for u in range(8):
                    emit_qk_unit(0, u, on_act=True)
                    if u % 2 == 1:
                        dummy_mms(3)
            with nc.named_scope("qkv_v"):
                for t in range(4):
                    emit_v_unit(t)
                    dummy_mms(2)
            # cover the stretch between the last DMA-paced unit and the
            # attention pipeline reaching steady state.
            dummy_mms(6)

            def qk_filler(p, u):
                return lambda: emit_qk_unit(p, u, on_act=True)

            # ---- phase 2: attention, with fillers between blocks ----
            sch_ctr = [0]

            for p in range(NP):
                qA = qk[2 * p]
                kA = qk[2 * p + 1]
                with nc.named_scope(f"attn{p}"):
                    for j in range(NCH):
                        nblk = 4 * j + 4

                        # filler distribution (14 / 14 / 12+4): each pair
                        # lazily emits its own next-chunk q/k units plus the
                        # next pair's early ones, so every chunk has PE work
                        # to cover exp latency and the normalize window.
                        fillers = []
                        if p == 0:
                            if j < 3:
                                fillers += [
                                    (lambda t=t: emit_v_unit(t))
                                    for t in range(4 * j + 4, 4 * j + 8)]
                            else:
                                fillers += [qk_filler(1, 0), qk_filler(1, 4)]
                        elif p == 1:
                            if j < 3:
                                fillers += [qk_filler(1, j + 1),
                                            qk_filler(1, j + 5)]
                                fillers += [qk_filler(2, j),
                                            qk_filler(2, j + 4)]
                            else:
                                fillers += [qk_filler(2, 3)]
                        else:
                            if j == 0:
                                fillers += [qk_filler(2, 7)]
                            else:
                                fillers += [
                                    (lambda t=t: emit_proj_t(t))
                                    for t in range(4 * (j - 1), 4 * j)]
                        nf = len(fillers)
                        emitted = [0]
                        hold = min(2, nf)   # keep some for the chunk end

                        def pump(i):
                            want = min(nf - hold, (i + 1) * (nf - hold) // max(nblk - 1, 1))
                            while emitted[0] < want:
                                fillers[emitted[0]]()
                                emitted[0] += 1

                        pvA = psV.tile([128, 512], f32, tag="pv")
                        pvB = psV.tile([128, 512], f32, tag="pv")
                        Ps = [None] * nblk
                        ms = [None] * nblk

                        def emit_S(i):
                            m = i - 4 * j
                            lo = 128 * m if m >= 0 else 0
                            sp = psA.tile([128, 1024], f32, tag="A")
                            for ab in range(2):
                                nc.tensor.matmul(
                                    sp[:, ab * 512 + lo:(ab + 1) * 512],
                                    lhsT=kA[ab * 64:(ab + 1) * 64,
                                            i * 128:(i + 1) * 128],
                                    rhs=qA[ab * 64:(ab + 1) * 64,
                                           j * 512 + lo:(j + 1) * 512],
                                    start=True, stop=(m < 0))
                            if m >= 0:
                                # one masking matmul covers both heads: the
                                # out AP picks the two 128-col diagonal
                                # sub-blocks (stride 512 apart).
                                dst2 = sp[:].rearrange(
                                    "p (a b) -> p a b", a=2)[:, :, lo:lo + 128]
                                nc.tensor.matmul(
                                    dst2, lhsT=nid[:], rhs=mk2[:],
                                    start=False, stop=True)
                            P = pP.tile([128, 1024], f16, tag="P")
                            if m < 0 and sch_ctr[0] % 3 == 2:
                                nc.vector.tensor_scalar(
                                    out=P[:].bitcast(i16), in0=sp[:],
                                    scalar1=SCH_A, scalar2=SCH_B,
                                    op0=Alu.mult, op1=Alu.add)
                            elif lo == 0:
                                nc.scalar.activation(P[:], sp[:], Exp)
                            else:
                                nc.scalar.activation(
                                    P[:, lo:1024], sp[:, lo:1024], Exp)
                            if m < 0:
                                sch_ctr[0] += 1
                            Ps[i], ms[i] = P, max(m, 0)

                        def emit_PV(i):
                            m = ms[i]
                            lo = 128 * m
                            P = Ps[i]
                            first, last = (i == 0), (i == nblk - 1)
                            nc.tensor.matmul(
                                pvA[:, lo:512], lhsT=vo[i][2 * p][:],
                                rhs=P[:, lo:512], start=first, stop=last)
                            nc.tensor.matmul(
                                pvB[:, lo:512], lhsT=vo[i][2 * p + 1][:],
                                rhs=P[:, 512 + lo:1024], start=first, stop=last)

                        # software-pipeline: S(i+1) before PV(i); fillers
                        # between blocks keep PE fed through exp latency.
                        emit_S(0)
                        for i in range(1, nblk):
                            emit_S(i)
                            emit_PV(i - 1)
                            pump(i)
                        emit_PV(nblk - 1)
                        pump(nblk)
                        while emitted[0] < nf:
                            fillers[emitted[0]]()
                            emitted[0] += 1
                        if p == 2 and j == 3:
                            # keep the PE warm through the final normalize
                            # so the last proj chunk runs at full clock.
                            dummy_mms(8)
                        elif p == 0 and j <= 1:
                            # pipeline-ramp coverage at the head
                            dummy_mms(3 - j)

                        # normalize. reciprocal_approx_fast only works at
                        # base partition 0, so: head B's denom (rows 0-63)
                        # is recip'd in place; head A's denom (rows 64-127)
                        # is staged, swapped down via DMA, then recip'd.
                        s1 = pst.tile([128, 512], f32, tag="st")
                        nc.vector.tensor_copy(s1[64:128, :], pvA[64:128, :])
                        nc.vector.reciprocal_approx_fast(
                            s1[0:64, :], pvB[0:64, :])
                        s2 = pst.tile([128, 512], f32, tag="rc")
                        nc.sync.dma_start(s2[0:64, :], s1[64:128, :])
                        nc.sync.dma_start(s2[64:128, :], s1[0:64, :])
                        s3 = pst.tile([64, 512], f32, tag="s3")
                        nc.vector.reciprocal_approx_fast(
                            s3[:], s2[0:64, :])
                        nc.vector.tensor_mul(
                            yt[p][0:64, j * 512:(j + 1) * 512],
                            pvA[0:64, :], s3[:])
                        nc.vector.tensor_mul(
                            yt[p][64:128, j * 512:(j + 1) * 512],
                            pvB[64:128, :], s2[64:128, :])

            # final proj chunk (needs pair-2 chunk 3's normalize)
            with nc.named_scope("proj"):
                for t in range(12, 16):
                    emit_proj_t(t)

    nc.compile()
    return nc


def _prep_inputs(x, W_qkv, b_qkv, W_proj):
    """Per-core input maps (numpy; all matmul operands float16)."""
    sc = 1.0 / np.sqrt(D)
    nid = (-30.0 * np.eye(128)).astype(np.float16)
    mkl = np.tril(np.ones((128, 128)), -1).astype(np.float16)
    in_maps = []
    for c in range(N_CORES):
        b, hh = c // 2, c % 2
        h0 = hh * 384                      # column offset of this half's heads
        wq = W_qkv[:, h0:h0 + 384] * sc
        wk = W_qkv[:, 768 + h0:768 + h0 + 384]
        wv = W_qkv[:, 1536 + h0:1536 + h0 + 384]
        wa = np.ascontiguousarray(
            np.concatenate([wq, wk, wv], axis=1), dtype=np.float16)
        m = {
            "xT": np.ascontiguousarray(x[b].T, dtype=np.float16),
            "wa": wa,
            "wp": np.ascontiguousarray(W_proj[h0:h0 + 384, :], np.float16),
            "nid": nid,
            "mkl": mkl,
        }
        if np.any(b_qkv):
            bq = b_qkv[h0:h0 + 384] * sc
            bk = b_qkv[768 + h0:768 + h0 + 384]
            bv = b_qkv[1536 + h0:1536 + h0 + 384]
            m["ox"] = np.ones((1, T), dtype=np.float16)
            m["wb"] = np.concatenate([bq, bk, bv]).reshape(1, 1152).astype(
                np.float16)
        in_maps.append(m)
    return in_maps


def _run(inputs, trace=False, tmpdir=None):
    from concourse.bass_utils import run_bass_kernel_spmd

    x = np.asarray(inputs["x"], dtype=np.float32)
    W_qkv = np.asarray(inputs["W_qkv"], dtype=np.float32)
    b_qkv = np.asarray(inputs["b_qkv"], dtype=np.float32)
    W_proj = np.asarray(inputs["W_proj"], dtype=np.float32)
    b_proj = np.asarray(inputs["b_proj"], dtype=np.float32)

    has_bias = bool(np.any(b_qkv))
    key = ("k", has_bias)
    if key not in _cache:
        _cache[key] = _build(has_bias)
    nc = _cache[key]

    in_maps = _prep_inputs(x, W_qkv, b_qkv, W_proj)
    res = run_bass_kernel_spmd(nc, in_maps, list(range(N_CORES)),
                               trace=trace, tmpdir=tmpdir)
    out = np.empty((B, T, C), dtype=np.float32)
    for b in range(B):
        out[b] = res.results[2 * b]["out"] + res.results[2 * b + 1]["out"]
    out += b_proj
    return out, res


def kernel(**inputs):
    out, _ = _run(inputs)
    return out
